# revision 57
# baseline (speedup 1.0000x reference)
"""Trainium2 Bass kernel for nn_FDN_88012469830490.

FDN reverb: IR synthesis (host, tiny 6x6 solves) + FFT convolution
(device) of x (16,2,441000) with the 2x2x88200 IR.

Device algorithm per core (2 batches/core, A=2c, B=2c+1):
  overlap-save conv, FFT N=262144 = 128*128*16, hop 173945, 3 blocks.
  Batch packing: V_i = FFT(x_i^A + j x_i^B) per channel i; spectral
  S_o = V_0 H_o0 + V_1 H_o1 (H = host FFT of IR, scaled 1/512);
  IFFT(S_o) = y_o^A + j y_o^B. 12 FFTs/core, all fp16 matmuls on PE
  with fp32 PSUM; twiddles t2 (fwd), tA and the n3-part of tB (inv)
  folded into 16 per-n3 stationary matrices; remaining twiddles (t1
  fwd, tB-residual inv) as fp16 TensorTensor cmuls (DVE 2x mode, one
  product on GpSimd); PSUM exits on ACT/DVE; output stored fp16 and
  widened on host. Stage-interleaved software pipeline: two FFT chains
  in flight (inv(b,o) interleaved with fwd(b+1,o)) over double-buffered
  tiles; x loads/consts split across SP/ACT DMA queues ahead of the
  big constant tables.

Layouts (digits: n = n1*2048+n2*16+n3, k = k1+128*k2+16384*k3,
k1 = 16*u+j):
  fwd: [n1; n2*16+n3] -S1-> [k1; m] -t1-> -T1-> [n2; n3*128+k1]
       -S2(t2 fold)-> [k2; n3*128+k1] -T2-> [n3*8+u; j*128+k2]
       -S3-> [k3*8+u; j*128+k2]
  inv: -S1'-> [n3*8+u; j*128+k2] -T2'-> [k2; j*128+n3*8+u]
       -S2'(tA fold)-> [n2; ...] -tBres-> -T1'-> [j*8+u; n3*128+n2]
       -S3'(n3-fold, 1/512)-> [n1; n3*128+n2] -perm-> [n1; n2*16+n3]
"""
import sys
import numpy as np

sys.path.insert(0, "/opt/trn_rl_repo")

# ---------------- problem constants ----------------
SR = 44100
DELAYS = np.array([997, 1153, 1327, 1559, 1801, 2099])
ND = 6
L = 88200
FB = L // 2 + 1
NDF = 49
T60 = 1.5
GAMMA_MAX = 10.0 ** ((-60.0 / SR / T60 * DELAYS) / 20.0)

T = 441000
N = 262144
P1 = 128
M2 = 2048
HOP = N - (L - 1)     # 173945
NBLK = 3
NCORES = 8

VROW, VCOL = 43, 135  # L-1 = 88199 = 43*2048 + 135
ROW_TAIL = M2 - VCOL              # 1913
NMAT = 51
NTAB = 2


# ---------------- host IR synthesis ----------------
def _expm_skew(S):
    lam, V = np.linalg.eigh(1j * S)
    return (V @ np.diag(np.exp(-1j * lam)) @ V.conj().T).real


def _host_ir(b, c, U_raw, gamma_raw):
    """IR h (2, 2, L) float64."""
    tri = np.triu(U_raw.astype(np.float64), 1)
    U = _expm_skew(tri - tri.T)
    gamma = (1.0 / (1.0 + np.exp(-gamma_raw.astype(np.float64)))) * GAMMA_MAX
    pos = np.arange(FB) * ((NDF - 1) / (FB - 1))
    i0 = np.clip(np.floor(pos).astype(int), 0, NDF - 2)
    frac = (pos - i0)[:, None]
    g = gamma[i0] * (1 - frac) + gamma[i0 + 1] * frac
    A = U[None, :, :] * g[:, None, :]
    freqs = np.arange(FB) / L * 2 * np.pi
    invD = np.exp(1j * freqs[:, None] * DELAYS)
    Mm = invD[:, :, None] * np.eye(ND) - A
    bc = np.broadcast_to(b.astype(np.float64), (FB, ND, 2))
    X = np.linalg.solve(Mm, bc)
    H = np.einsum('ci,fio->fco', c.astype(complex), X)
    h = np.fft.irfft(H.transpose(1, 2, 0), n=L)
    return h


def _kmap():
    p = np.arange(128)[:, None]
    f = np.arange(M2)[None, :]
    k3, u = p // 8, p % 8
    j, k2 = f // 128, f % 128
    k1 = 16 * u + j
    return k1 + 128 * k2 + 16384 * k3


def _host_htab(h):
    """H tables (2,2,2,128,2048) fp16: [o,i,(r,i)] spectra / 512."""
    km = _kmap()
    out = np.empty((128, 8 * M2), np.float16)
    for o in range(2):
        for i in range(2):
            hp = np.zeros(N)
            hp[:L] = h[o, i]
            Hf = np.fft.fft(hp) / 512.0
            idx = (o * 2 + i) * 2
            out[:, idx * M2:(idx + 1) * M2] = Hf[km].real.astype(np.float16)
            out[:, (idx + 1) * M2:(idx + 2) * M2] = Hf[km].imag.astype(np.float16)
    return out


def _consts():
    """Stationary matrices + twiddle tables, host-packed partition-major.

    mats_packed: (128, NMAT*3*128) fp16 — mat idx m, comp k (r/i/-i):
      cols [ (m*3+k)*128 : +128 ].
    tabs_packed: (128, NTAB*2*2048) fp16 — t1(r,i), tBres(r,i).
    """
    q = np.arange(128)
    k1q = 16 * (q % 8) + q // 8

    mats = np.zeros((NMAT, 128, 128), complex)
    # 0: F128 fwd (S1): F[k1, n1] = W^-
    mats[0] = np.exp(-2j * np.pi * np.outer(np.arange(128), np.arange(128)) / 128)
    # 1: Btil fwd (S3): [k3*8+u, n3*8+u] = exp(-2pi i n3 k3/16)
    for u in range(8):
        n3g, k3g = np.meshgrid(np.arange(16), np.arange(16), indexing='xy')
        mats[1][k3g * 8 + u, n3g * 8 + u] = np.exp(-2j * np.pi * n3g * k3g / 16)
        # 2: Btil' inv (S1'): [n3*8+u, k3*8+u] = exp(+2pi i n3 k3/16)
        mats[2][n3g * 8 + u, k3g * 8 + u] = np.exp(2j * np.pi * n3g * k3g / 16)
    # 3..18: fwd S2 with t2 fold: M[n3][k2, n2] = W2048^{-k2 n3} * F128^-[k2,n2]
    F = np.exp(-2j * np.pi * np.outer(np.arange(128), np.arange(128)) / 128)
    for n3 in range(16):
        d = np.exp(-2j * np.pi * np.arange(128) * n3 / M2)
        mats[3 + n3] = d[:, None] * F
    # 19..34: inv S2' with tA fold: A[n3][n2, k2] = F128^+[n2,k2] * W2048^{+n3 k2}
    Fp = np.exp(2j * np.pi * np.outer(np.arange(128), np.arange(128)) / 128)
    for n3 in range(16):
        d = np.exp(2j * np.pi * np.arange(128) * n3 / M2)
        mats[19 + n3] = Fp * d[None, :]
    # 35..50: inv S3' with n3-part of tB + 1/512:
    # P[n3][n1, q] = exp(+2pi i k1(q) (2048*n1 + n3)/N) / 512
    for n3 in range(16):
        mats[35 + n3] = np.exp(
            2j * np.pi * k1q[None, :] * (2048 * np.arange(128)[:, None] + n3) / N
        ) / 512.0

    # matmul computes lhsT.T @ rhs -> store each stationary TRANSPOSED
    mats_packed = np.empty((128, NMAT * 3 * 128), np.float16)
    for m in range(NMAT):
        mt = mats[m].T
        mats_packed[:, (m * 3 + 0) * 128:(m * 3 + 1) * 128] = mt.real.astype(np.float16)
        mats_packed[:, (m * 3 + 1) * 128:(m * 3 + 2) * 128] = mt.imag.astype(np.float16)
        mats_packed[:, (m * 3 + 2) * 128:(m * 3 + 3) * 128] = (-mt.imag).astype(np.float16)

    # tables
    t1 = np.exp(-2j * np.pi * np.outer(np.arange(128), np.arange(M2)) / N)
    # tBres layout [n2; n3*128 + j*8 + u], k1 = 16u + j
    f = np.arange(M2)
    rem = f % 128
    jf, uf = rem // 8, rem % 8
    k1f = 16 * uf + jf
    tb = np.exp(2j * np.pi * np.outer(np.arange(128), k1f) / 16384.0)
    tabs_packed = np.empty((128, NTAB * 2 * M2), np.float16)
    tabs_packed[:, 0*M2:1*M2] = t1.real.astype(np.float16)
    tabs_packed[:, 1*M2:2*M2] = t1.imag.astype(np.float16)
    tabs_packed[:, 2*M2:3*M2] = tb.real.astype(np.float16)
    tabs_packed[:, 3*M2:4*M2] = tb.imag.astype(np.float16)

    ident = np.eye(128, dtype=np.float16)
    return mats_packed, tabs_packed, ident


# ---------------- bass program ----------------
_PROG = None


def _build_program():
    import concourse.bass as bass
    import concourse.tile as tile
    from concourse import bacc, mybir

    f32 = mybir.dt.float32
    f16 = mybir.dt.float16
    alu = mybir.AluOpType
    nc = bacc.Bacc("TRN2", target_bir_lowering=False, debug=False,
                   enable_asserts=False, num_devices=NCORES)

    import os
    kdbg = bool(os.environ.get("KDBG"))
    xp = nc.dram_tensor("xp", [2, 2, T], f16, kind="ExternalInput").ap()
    if kdbg:
        dbgh = nc.dram_tensor("dbgh", [16, 128, M2], f16, kind="ExternalOutput").ap()
    mats_d = nc.dram_tensor("mats", [128, NMAT * 3 * 128], f16, kind="ExternalInput").ap()
    tabs_d = nc.dram_tensor("tabs", [128, NTAB * 2 * M2], f16, kind="ExternalInput").ap()
    htab_d = nc.dram_tensor("htab", [128, 8 * M2], f16, kind="ExternalInput").ap()
    id_d = nc.dram_tensor("ident", [128, 128], f16, kind="ExternalInput").ap()
    zeros_d = nc.dram_tensor("zeros", [128, M2], f16, kind="ExternalInput").ap()
    yp = nc.dram_tensor("yp", [2, 2, T], f16, kind="ExternalOutput").ap()

    CW = 512

    from contextlib import ExitStack
    with tile.TileContext(nc) as tc, ExitStack() as ctx:
        cpool = ctx.enter_context(tc.tile_pool(name="consts", bufs=1))
        work = ctx.enter_context(tc.tile_pool(name="work", bufs=1))
        psA = ctx.enter_context(tc.tile_pool(name="psA", bufs=2, space="PSUM"))
        psT = ctx.enter_context(tc.tile_pool(name="psT", bufs=1, space="PSUM"))

        mats = cpool.tile([128, NMAT * 3 * 128], f16, tag="mats")
        tabs = cpool.tile([128, NTAB * 2 * M2], f16, tag="tabs")
        htt = cpool.tile([128, 8 * M2], f16, tag="htt")
        idt = cpool.tile([128, 128], f16, tag="idt")
        nc.sync.dma_start(mats[:, 0:3 * 128], mats_d[:, 0:3 * 128])

        def load_consts_small():
            nc.scalar.dma_start(tabs[:], tabs_d[:, :])
            nc.scalar.dma_start(idt[:], id_d[:, :])

        def load_consts_rest():
            nc.sync.dma_start(mats[:, 3 * 128:9 * 3 * 128],
                              mats_d[:, 3 * 128:9 * 3 * 128])
            nc.sync.dma_start(mats[:, 9 * 3 * 128:], mats_d[:, 9 * 3 * 128:])
            nc.scalar.dma_start(htt[:], htab_d[:, :])

        def MAT(m, k):
            return mats[:, (m * 3 + k) * 128:(m * 3 + k + 1) * 128]

        def TAB(t, k):
            return tabs[:, (t * 2 + k) * M2:(t * 2 + k + 1) * M2]

        def HT(o, i, k):
            idx = ((o * 2 + i) * 2 + k)
            return htt[:, idx * M2:(idx + 1) * M2]

        def cmm(pr, pi, m, vr, vi, start, stop):
            """psum += M @ (vr + j vi), complex; M = mats[m] (r/i/-i)."""
            nc.tensor.matmul(pr, MAT(m, 0), vr, start=start, stop=False)
            nc.tensor.matmul(pr, MAT(m, 2), vi, start=False, stop=stop)
            nc.tensor.matmul(pi, MAT(m, 1), vr, start=start, stop=False)
            nc.tensor.matmul(pi, MAT(m, 0), vi, start=False, stop=stop)

        def stt(eng, out, a, b, op):
            """out = a op b. Plain TensorTensor: DVE gets 2x_1p in fp16
            (scalar_tensor_tensor would disable all DVE perf modes)."""
            if op is alu.mult:
                eng.tensor_mul(out, a, b)
            elif op is alu.add:
                eng.tensor_add(out, a, b)
            else:
                eng.tensor_sub(out, a, b)

        # engine rotation for PSUM-exit chunk copies
        cp_state = [0]

        def chunk_copy(dst, src, eng=None):
            """PSUM->SBUF chunk copy, rotating ACT (5) : DVE (1)."""
            i = cp_state[0]
            cp_state[0] += 1
            if i % 9 < 8:
                nc.scalar.copy(dst, src)
            else:
                nc.vector.tensor_copy(dst, src)

        def stage_plain(dst_r, dst_i, src_r, src_i, m):
            """dst = mats[m] @ src (complex), chunked 512; plain copy out."""
            for ch in range(4):
                sl = (slice(None), slice(ch * CW, (ch + 1) * CW))
                pr = psA.tile([128, CW], f32, tag="pr")
                pi = psA.tile([128, CW], f32, tag="pi")
                cmm(pr[:], pi[:], m, src_r[sl], src_i[sl], True, True)
                chunk_copy(dst_r[sl], pr[:])
                chunk_copy(dst_i[sl], pi[:])

        def stage_fold128(dst_r, dst_i, src_r, src_i, m0):
            """dst chunk n3 (128 wide, contiguous) = mats[m0+n3] @ src chunk."""
            for g in range(4):
                pr = psA.tile([128, CW], f32, tag="pr")
                pi = psA.tile([128, CW], f32, tag="pi")
                for q in range(4):
                    n3 = g * 4 + q
                    ssl = (slice(None), slice(n3 * 128, (n3 + 1) * 128))
                    psl = (slice(None), slice(q * 128, (q + 1) * 128))
                    cmm(pr[psl], pi[psl], m0 + n3, src_r[ssl], src_i[ssl],
                        True, True)
                sl = (slice(None), slice(g * CW, (g + 1) * CW))
                chunk_copy(dst_r[sl], pr[:])
                chunk_copy(dst_i[sl], pi[:])

        def cmul(dst_r, dst_i, sr, si, twr, twi):
            """dst = (sr + j si) * (twr + j twi), full-width fp16 STT."""
            for h in range(2):
                s = slice(h * 1024, (h + 1) * 1024)
                c0 = work.tile([128, M2], f16, tag="c0")
                c1 = work.tile([128, M2], f16, tag="c1")
                c3 = work.tile([128, M2], f16, tag="s0")
                stt(nc.gpsimd, c3[:, s], si[:, s], twr[:, s], alu.mult)
                stt(nc.vector, c0[:, s], sr[:, s], twr[:, s], alu.mult)
                stt(nc.vector, c1[:, s], si[:, s], twi[:, s], alu.mult)
                stt(nc.vector, dst_r[:, s], c0[:, s], c1[:, s], alu.subtract)
                stt(nc.vector, c0[:, s], sr[:, s], twi[:, s], alu.mult)
                stt(nc.vector, dst_i[:, s], c0[:, s], c3[:, s], alu.add)

        def stage_cmul(dst_r, dst_i, src_r, src_i, m, tw):
            """dst = tw * (mats[m] @ src): matmul, ACT precopy, STT cmul."""
            sr = work.tile([128, M2], f16, tag="sr", bufs=2)
            si = work.tile([128, M2], f16, tag="si", bufs=2)
            for ch in range(4):
                sl = (slice(None), slice(ch * CW, (ch + 1) * CW))
                pr = psA.tile([128, CW], f32, tag="pr")
                pi = psA.tile([128, CW], f32, tag="pi")
                cmm(pr[:], pi[:], m, src_r[sl], src_i[sl], True, True)
                nc.scalar.copy(sr[sl], pr[:])
                nc.scalar.copy(si[sl], pi[:])
            cmul(dst_r, dst_i, sr, si, tw[0], tw[1])

        def cmul_res(dst_r, dst_i, src_r, src_i, m0, tw):
            """inv S2' (tA folded, contiguous n3 chunks) + residual tB cmul."""
            sr = work.tile([128, M2], f16, tag="sr", bufs=2)
            si = work.tile([128, M2], f16, tag="si", bufs=2)
            stage_fold128(sr, si, src_r, src_i, m0)
            cmul(dst_r, dst_i, sr, si, tw[0], tw[1])

        def back_plain(d, pt, hf):
            if hf == 0:
                nc.vector.tensor_copy(d[:, 0:1024], pt[:])
            else:
                nc.scalar.copy(d[:, 1024:M2], pt[:])

        def back_ju(d, pt, hf):
            # psum [k2; j*128 + n3*8+u] -> dst [k2; n3*128 + j*8 + u]
            dd = d[:, :].rearrange("p (n j u) -> p j n u", n=16, j=16, u=8)
            pp = pt[:].rearrange("p (j n u) -> p j n u", j=8, n=16, u=8)
            if hf == 0:
                nc.vector.tensor_copy(dd[:, 0:8], pp)
            else:
                nc.scalar.copy(dd[:, 8:16], pp)

        def tpose16(dst_r, dst_i, src_r, src_i, slicer, back=back_plain):
            """16 PE transposes per plane -> fp16 psum -> 1 copyback.
            Real plane back on DVE, imag plane on ACT (parallel)."""
            for s, d in ((src_r, dst_r), (src_i, dst_i)):
                for hf in range(2):
                    pt = psT.tile([128, 1024], f16, tag="pt", bufs=4,
                                  name="pt_t")
                    for c in range(8):
                        cc = hf * 8 + c
                        nc.tensor.transpose(
                            pt[:, c * 128:(c + 1) * 128], slicer(s, cc), idt[:])
                    back(d, pt, hf)

        def sl_str16(s, c):          # fwd T1 / fwd T2: strided 16
            return s[:, c:M2:16]

        def sl_cont(s, c):           # inv T2' / inv T1': contiguous
            return s[:, c * 128:(c + 1) * 128]

        def fwd_stages(in_r, in_i, vr, vi):
            st = {}

            def s1():
                st['ar'] = work.tile([128, M2], f16, tag="ar", bufs=2, name="ar_t")
                st['ai'] = work.tile([128, M2], f16, tag="ai", bufs=2, name="ai_t")
                stage_cmul(st['ar'], st['ai'], in_r, in_i, 0,
                           (TAB(0, 0), TAB(0, 1)))

            def t1():
                st['btr'] = work.tile([128, M2], f16, tag="btr", bufs=2, name="btr_t")
                st['bti'] = work.tile([128, M2], f16, tag="bti", bufs=2, name="bti_t")
                tpose16(st['btr'], st['bti'], st['ar'], st['ai'], sl_str16)

            def s2():
                st['ar2'] = work.tile([128, M2], f16, tag="ar", bufs=2, name="ar_t")
                st['ai2'] = work.tile([128, M2], f16, tag="ai", bufs=2, name="ai_t")
                stage_fold128(st['ar2'], st['ai2'], st['btr'], st['bti'], 3)

            def t2():
                st['btr2'] = work.tile([128, M2], f16, tag="btr", bufs=2, name="btr_t")
                st['bti2'] = work.tile([128, M2], f16, tag="bti", bufs=2, name="bti_t")
                tpose16(st['btr2'], st['bti2'], st['ar2'], st['ai2'], sl_str16)

            def s3():
                stage_plain(vr, vi, st['btr2'], st['bti2'], 1)

            return [s1, t1, s2, t2, s3]

        def inv_stages(vt, blk, o):
            st = {}

            def sp():
                ssr = work.tile([128, M2], f16, tag="ssr", name="ssr_t")
                ssi = work.tile([128, M2], f16, tag="ssi", name="ssi_t")
                spectral(ssr, ssi, vt, o)
                st['ssr'], st['ssi'] = ssr, ssi

            def s1p():
                st['ar'] = work.tile([128, M2], f16, tag="ar", bufs=2, name="ar_t")
                st['ai'] = work.tile([128, M2], f16, tag="ai", bufs=2, name="ai_t")
                stage_plain(st['ar'], st['ai'], st['ssr'], st['ssi'], 2)

            def t2p():
                st['btr'] = work.tile([128, M2], f16, tag="btr", bufs=2, name="btr_t")
                st['bti'] = work.tile([128, M2], f16, tag="bti", bufs=2, name="bti_t")
                tpose16(st['btr'], st['bti'], st['ar'], st['ai'], sl_cont,
                        back=back_ju)

            def s2p():
                st['ar2'] = work.tile([128, M2], f16, tag="ar", bufs=2, name="ar_t")
                st['ai2'] = work.tile([128, M2], f16, tag="ai", bufs=2, name="ai_t")
                cmul_res(st['ar2'], st['ai2'], st['btr'], st['bti'], 19,
                         (TAB(1, 0), TAB(1, 1)))

            def t1p():
                st['btr2'] = work.tile([128, M2], f16, tag="btr", bufs=2, name="btr_t")
                st['bti2'] = work.tile([128, M2], f16, tag="bti", bufs=2, name="bti_t")
                tpose16(st['btr2'], st['bti2'], st['ar2'], st['ai2'], sl_cont)

            def s3p():
                out_r = work.tile([128, M2], f16, tag="outr")
                out_i = work.tile([128, M2], f16, tag="outi")
                bt_r2, bt_i2 = st['btr2'], st['bti2']
                for g in range(4):
                    pr = psA.tile([128, CW], f32, tag="pr")
                    pi = psA.tile([128, CW], f32, tag="pi")
                    for q in range(4):
                        n3 = g * 4 + q
                        ssl = (slice(None), slice(n3 * 128, (n3 + 1) * 128))
                        psl = (slice(None), slice(q * 128, (q + 1) * 128))
                        cmm(pr[psl], pi[psl], 35 + n3, bt_r2[ssl], bt_i2[ssl],
                            True, True)
                    for d, p in ((out_r, pr), (out_i, pi)):
                        dd = d[:, :].rearrange("p (n2 n3) -> p n3 n2", n3=16)
                        ps = p[:].rearrange("p (q n2) -> p q n2", q=4)
                        nc.scalar.copy(dd[:, g*4:g*4+4, :], ps)
                store_block(out_r, yp[0, o], blk)
                store_block(out_i, yp[1, o], blk)

            return [sp, s1p, t2p, s2p, t1p, s3p]

        def load_block(t_, src, blk, eng):
            if blk == 0:
                eng.dma_start(t_[0:VROW + 1, :], zeros_d[0:VROW + 1, :])
                eng.dma_start(
                    t_[VROW:VROW + 1, VCOL:M2],
                    src[0:ROW_TAIL].rearrange('(a b) -> a b', a=1))
                eng.dma_start(
                    t_[VROW + 1:128, :],
                    src[ROW_TAIL:HOP].rearrange("(r m) -> r m", m=M2))
            elif blk == 1:
                s0 = HOP - (L - 1)
                eng.dma_start(
                    t_[:, :], src[s0:s0 + N].rearrange("(r m) -> r m", m=M2))
            else:
                s0 = 2 * HOP - (L - 1)
                nfull = (T - s0) // M2            # 88
                rem = (T - s0) - nfull * M2       # 1085
                eng.dma_start(t_[nfull:128, :], zeros_d[nfull:128, :])
                eng.dma_start(
                    t_[0:nfull, :],
                    src[s0:s0 + nfull * M2].rearrange("(r m) -> r m", m=M2))
                eng.dma_start(
                    t_[nfull:nfull + 1, 0:rem],
                    src[s0 + nfull * M2:T].rearrange('(a b) -> a b', a=1))

        def store_block(out_t, dst, blk):
            base = blk * HOP
            nc.scalar.dma_start(
                dst[base:base + ROW_TAIL].rearrange('(a b) -> a b', a=1),
                out_t[VROW:VROW + 1, VCOL:M2])
            if blk < 2:
                nc.scalar.dma_start(
                    dst[base + ROW_TAIL:base + HOP].rearrange("(r m) -> r m", m=M2),
                    out_t[VROW + 1:128, :])
            else:
                nrem = T - base - ROW_TAIL
                nfull = nrem // M2                # 44
                rem = nrem - nfull * M2           # 1085
                nc.scalar.dma_start(
                    dst[base + ROW_TAIL:base + ROW_TAIL + nfull * M2]
                        .rearrange("(r m) -> r m", m=M2),
                    out_t[VROW + 1:VROW + 1 + nfull, :])
                nc.scalar.dma_start(
                    dst[T - rem:T].rearrange('(a b) -> a b', a=1),
                    out_t[VROW + 1 + nfull:VROW + 2 + nfull, 0:rem])

        def spectral(sr_, si_, v, o):
            """S_o = V0*H_o0 + V1*H_o1 (fp16, SBUF), accumulated in place."""
            (v0r, v0i), (v1r, v1i) = v
            for h in range(2):
                s = slice(h * 1024, (h + 1) * 1024)
                # real part
                c0 = work.tile([128, M2], f16, tag="s0")
                c1 = work.tile([128, M2], f16, tag="s1")
                stt(nc.gpsimd, c1[:, s], v1i[:, s], HT(o, 1, 1)[:, s], alu.mult)
                stt(nc.vector, sr_[:, s], v0r[:, s], HT(o, 0, 0)[:, s], alu.mult)
                stt(nc.vector, c0[:, s], v0i[:, s], HT(o, 0, 1)[:, s], alu.mult)
                stt(nc.vector, sr_[:, s], sr_[:, s], c0[:, s], alu.subtract)
                stt(nc.vector, c0[:, s], v1r[:, s], HT(o, 1, 0)[:, s], alu.mult)
                stt(nc.vector, sr_[:, s], sr_[:, s], c0[:, s], alu.add)
                stt(nc.vector, sr_[:, s], sr_[:, s], c1[:, s], alu.subtract)
                # imag part
                stt(nc.gpsimd, c1[:, s], v1i[:, s], HT(o, 1, 0)[:, s], alu.mult)
                stt(nc.vector, si_[:, s], v0r[:, s], HT(o, 0, 1)[:, s], alu.mult)
                stt(nc.vector, c0[:, s], v0i[:, s], HT(o, 0, 0)[:, s], alu.mult)
                stt(nc.vector, si_[:, s], si_[:, s], c0[:, s], alu.add)
                stt(nc.vector, c0[:, s], v1r[:, s], HT(o, 1, 1)[:, s], alu.mult)
                stt(nc.vector, si_[:, s], si_[:, s], c0[:, s], alu.add)
                stt(nc.vector, si_[:, s], si_[:, s], c1[:, s], alu.add)

        def dbg_dump(idx, t_):
            if kdbg:
                nc.sync.dma_start(dbgh[idx], t_[:, :])

        # ---- program: stage-interleaved software pipeline ----
        def mk_fwd(blk, i):
            in_r = work.tile([128, M2], f16, tag="inr", bufs=2)
            in_i = work.tile([128, M2], f16, tag="ini", bufs=2)
            ldq = nc.sync if i == 0 else nc.scalar
            load_block(in_r, xp[0, i], blk, ldq)
            load_block(in_i, xp[1, i], blk, ldq)
            vb = 2 if i == 0 else 1
            vr = work.tile([128, M2], f16, tag=f"v{i}r", bufs=vb)
            vi = work.tile([128, M2], f16, tag=f"v{i}i", bufs=vb)
            return fwd_stages(in_r, in_i, vr, vi), (vr, vi)

        def run2(a, b):
            """Interleave two stage lists (either may be None)."""
            a = a or []
            b = b or []
            n = max(len(a), len(b))
            for k in range(n):
                if k < len(a):
                    a[k]()
                if k < len(b):
                    b[k]()

        fa, va = mk_fwd(0, 0)
        load_consts_small()
        fb, vb_ = mk_fwd(0, 1)
        load_consts_rest()
        run2(fa, fb)
        vt = [va, vb_]
        for blk in range(NBLK):
            nxt = blk + 1 < NBLK
            if nxt:
                f0, nv0 = mk_fwd(blk + 1, 0)
                run2(inv_stages(vt, blk, 0), f0)
                f1, nv1 = mk_fwd(blk + 1, 1)
                run2(inv_stages(vt, blk, 1), f1)
                vt = [nv0, nv1]
            else:
                run2(inv_stages(vt, blk, 0), inv_stages(vt, blk, 1))

    nc.compile()
    return nc


def _get_prog():
    global _PROG
    if _PROG is None:
        _PROG = _build_program()
    return _PROG


# ---------------- public entry ----------------
def kernel(x, b, c, U_raw, gamma_raw):
    from concourse import bass_utils

    x16 = np.ascontiguousarray(np.asarray(x).astype(np.float16))
    h = _host_ir(np.asarray(b, np.float64), np.asarray(c, np.float64),
                 np.asarray(U_raw, np.float64), np.asarray(gamma_raw, np.float64))
    htab = _host_htab(h)
    mats_packed, tabs_packed, ident = _consts()
    nc = _get_prog()

    in_maps = []
    for core in range(NCORES):
        in_maps.append({
            "xp": x16[2 * core:2 * core + 2],
            "mats": mats_packed, "tabs": tabs_packed,
            "htab": htab, "ident": ident,
            "zeros": np.zeros((128, M2), np.float16),
        })
    res = bass_utils.run_bass_kernel_spmd(nc, in_maps, core_ids=list(range(NCORES)))
    y = np.empty((16, 2, T), np.float32)
    for core in range(NCORES):
        y[2 * core:2 * core + 2] = res.results[core]["yp"].astype(np.float32)
    return y


# revision 58
# speedup vs baseline: 1.0018x; 1.0018x over previous
"""Trainium2 Bass kernel for nn_FDN_88012469830490.

FDN reverb: IR synthesis (host, tiny 6x6 solves) + FFT convolution
(device) of x (16,2,441000) with the 2x2x88200 IR.

Device algorithm per core (2 batches/core, A=2c, B=2c+1):
  overlap-save conv, FFT N=262144 = 128*128*16, hop 173945, 3 blocks.
  Batch packing: V_i = FFT(x_i^A + j x_i^B) per channel i; spectral
  S_o = V_0 H_o0 + V_1 H_o1 (H = host FFT of IR, scaled 1/512);
  IFFT(S_o) = y_o^A + j y_o^B. 12 FFTs/core, all fp16 matmuls on PE
  with fp32 PSUM; twiddles t2 (fwd), tA and the n3-part of tB (inv)
  folded into 16 per-n3 stationary matrices; remaining twiddles (t1
  fwd, tB-residual inv) as fp16 TensorTensor cmuls (DVE 2x mode, one
  product on GpSimd); PSUM exits on ACT/DVE; output stored fp16 and
  widened on host. Stage-interleaved software pipeline: two FFT chains
  in flight (inv(b,o) interleaved with fwd(b+1,o)) over double-buffered
  tiles; x loads/consts split across SP/ACT DMA queues ahead of the
  big constant tables.

Layouts (digits: n = n1*2048+n2*16+n3, k = k1+128*k2+16384*k3,
k1 = 16*u+j):
  fwd: [n1; n2*16+n3] -S1-> [k1; m] -t1-> -T1-> [n2; n3*128+k1]
       -S2(t2 fold)-> [k2; n3*128+k1] -T2-> [n3*8+u; j*128+k2]
       -S3-> [k3*8+u; j*128+k2]
  inv: -S1'-> [n3*8+u; j*128+k2] -T2'-> [k2; j*128+n3*8+u]
       -S2'(tA fold)-> [n2; ...] -tBres-> -T1'-> [j*8+u; n3*128+n2]
       -S3'(n3-fold, 1/512)-> [n1; n3*128+n2] -perm-> [n1; n2*16+n3]
"""
import sys
import numpy as np

sys.path.insert(0, "/opt/trn_rl_repo")

# ---------------- problem constants ----------------
SR = 44100
DELAYS = np.array([997, 1153, 1327, 1559, 1801, 2099])
ND = 6
L = 88200
FB = L // 2 + 1
NDF = 49
T60 = 1.5
GAMMA_MAX = 10.0 ** ((-60.0 / SR / T60 * DELAYS) / 20.0)

T = 441000
N = 262144
P1 = 128
M2 = 2048
HOP = N - (L - 1)     # 173945
NBLK = 3
NCORES = 8

VROW, VCOL = 43, 135  # L-1 = 88199 = 43*2048 + 135
ROW_TAIL = M2 - VCOL              # 1913
NMAT = 51
NTAB = 2


# ---------------- host IR synthesis ----------------
def _expm_skew(S):
    lam, V = np.linalg.eigh(1j * S)
    return (V @ np.diag(np.exp(-1j * lam)) @ V.conj().T).real


def _host_ir(b, c, U_raw, gamma_raw):
    """IR h (2, 2, L) float64."""
    tri = np.triu(U_raw.astype(np.float64), 1)
    U = _expm_skew(tri - tri.T)
    gamma = (1.0 / (1.0 + np.exp(-gamma_raw.astype(np.float64)))) * GAMMA_MAX
    pos = np.arange(FB) * ((NDF - 1) / (FB - 1))
    i0 = np.clip(np.floor(pos).astype(int), 0, NDF - 2)
    frac = (pos - i0)[:, None]
    g = gamma[i0] * (1 - frac) + gamma[i0 + 1] * frac
    A = U[None, :, :] * g[:, None, :]
    freqs = np.arange(FB) / L * 2 * np.pi
    invD = np.exp(1j * freqs[:, None] * DELAYS)
    Mm = invD[:, :, None] * np.eye(ND) - A
    bc = np.broadcast_to(b.astype(np.float64), (FB, ND, 2))
    X = np.linalg.solve(Mm, bc)
    H = np.einsum('ci,fio->fco', c.astype(complex), X)
    h = np.fft.irfft(H.transpose(1, 2, 0), n=L)
    return h


def _kmap():
    p = np.arange(128)[:, None]
    f = np.arange(M2)[None, :]
    k3, u = p // 8, p % 8
    j, k2 = f // 128, f % 128
    k1 = 16 * u + j
    return k1 + 128 * k2 + 16384 * k3


def _host_htab(h):
    """H tables (2,2,2,128,2048) fp16: [o,i,(r,i)] spectra / 512."""
    km = _kmap()
    out = np.empty((128, 8 * M2), np.float16)
    for o in range(2):
        for i in range(2):
            hp = np.zeros(N)
            hp[:L] = h[o, i]
            Hf = np.fft.fft(hp) / 512.0
            idx = (o * 2 + i) * 2
            out[:, idx * M2:(idx + 1) * M2] = Hf[km].real.astype(np.float16)
            out[:, (idx + 1) * M2:(idx + 2) * M2] = Hf[km].imag.astype(np.float16)
    return out


def _consts():
    """Stationary matrices + twiddle tables, host-packed partition-major.

    mats_packed: (128, NMAT*3*128) fp16 — mat idx m, comp k (r/i/-i):
      cols [ (m*3+k)*128 : +128 ].
    tabs_packed: (128, NTAB*2*2048) fp16 — t1(r,i), tBres(r,i).
    """
    q = np.arange(128)
    k1q = 16 * (q % 8) + q // 8

    mats = np.zeros((NMAT, 128, 128), complex)
    # 0: F128 fwd (S1): F[k1, n1] = W^-
    mats[0] = np.exp(-2j * np.pi * np.outer(np.arange(128), np.arange(128)) / 128)
    # 1: Btil fwd (S3): [k3*8+u, n3*8+u] = exp(-2pi i n3 k3/16)
    for u in range(8):
        n3g, k3g = np.meshgrid(np.arange(16), np.arange(16), indexing='xy')
        mats[1][k3g * 8 + u, n3g * 8 + u] = np.exp(-2j * np.pi * n3g * k3g / 16)
        # 2: Btil' inv (S1'): [n3*8+u, k3*8+u] = exp(+2pi i n3 k3/16)
        mats[2][n3g * 8 + u, k3g * 8 + u] = np.exp(2j * np.pi * n3g * k3g / 16)
    # 3..18: fwd S2 with t2 fold: M[n3][k2, n2] = W2048^{-k2 n3} * F128^-[k2,n2]
    F = np.exp(-2j * np.pi * np.outer(np.arange(128), np.arange(128)) / 128)
    for n3 in range(16):
        d = np.exp(-2j * np.pi * np.arange(128) * n3 / M2)
        mats[3 + n3] = d[:, None] * F
    # 19..34: inv S2' with tA fold: A[n3][n2, k2] = F128^+[n2,k2] * W2048^{+n3 k2}
    Fp = np.exp(2j * np.pi * np.outer(np.arange(128), np.arange(128)) / 128)
    for n3 in range(16):
        d = np.exp(2j * np.pi * np.arange(128) * n3 / M2)
        mats[19 + n3] = Fp * d[None, :]
    # 35..50: inv S3' with n3-part of tB + 1/512:
    # P[n3][n1, q] = exp(+2pi i k1(q) (2048*n1 + n3)/N) / 512
    for n3 in range(16):
        mats[35 + n3] = np.exp(
            2j * np.pi * k1q[None, :] * (2048 * np.arange(128)[:, None] + n3) / N
        ) / 512.0

    # matmul computes lhsT.T @ rhs -> store each stationary TRANSPOSED
    mats_packed = np.empty((128, NMAT * 3 * 128), np.float16)
    for m in range(NMAT):
        mt = mats[m].T
        mats_packed[:, (m * 3 + 0) * 128:(m * 3 + 1) * 128] = mt.real.astype(np.float16)
        mats_packed[:, (m * 3 + 1) * 128:(m * 3 + 2) * 128] = mt.imag.astype(np.float16)
        mats_packed[:, (m * 3 + 2) * 128:(m * 3 + 3) * 128] = (-mt.imag).astype(np.float16)

    # tables
    t1 = np.exp(-2j * np.pi * np.outer(np.arange(128), np.arange(M2)) / N)
    # tBres layout [n2; n3*128 + j*8 + u], k1 = 16u + j
    f = np.arange(M2)
    rem = f % 128
    jf, uf = rem // 8, rem % 8
    k1f = 16 * uf + jf
    tb = np.exp(2j * np.pi * np.outer(np.arange(128), k1f) / 16384.0)
    tabs_packed = np.empty((128, NTAB * 2 * M2), np.float16)
    tabs_packed[:, 0*M2:1*M2] = t1.real.astype(np.float16)
    tabs_packed[:, 1*M2:2*M2] = t1.imag.astype(np.float16)
    tabs_packed[:, 2*M2:3*M2] = tb.real.astype(np.float16)
    tabs_packed[:, 3*M2:4*M2] = tb.imag.astype(np.float16)

    ident = np.eye(128, dtype=np.float16)
    return mats_packed, tabs_packed, ident


# ---------------- bass program ----------------
_PROG = None


def _build_program():
    import concourse.bass as bass
    import concourse.tile as tile
    from concourse import bacc, mybir

    f32 = mybir.dt.float32
    f16 = mybir.dt.float16
    alu = mybir.AluOpType
    nc = bacc.Bacc("TRN2", target_bir_lowering=False, debug=False,
                   enable_asserts=False, num_devices=NCORES)

    import os
    kdbg = bool(os.environ.get("KDBG"))
    xp = nc.dram_tensor("xp", [2, 2, T], f16, kind="ExternalInput").ap()
    if kdbg:
        dbgh = nc.dram_tensor("dbgh", [16, 128, M2], f16, kind="ExternalOutput").ap()
    mats_d = nc.dram_tensor("mats", [128, NMAT * 3 * 128], f16, kind="ExternalInput").ap()
    tabs_d = nc.dram_tensor("tabs", [128, NTAB * 2 * M2], f16, kind="ExternalInput").ap()
    htab_d = nc.dram_tensor("htab", [128, 8 * M2], f16, kind="ExternalInput").ap()
    id_d = nc.dram_tensor("ident", [128, 128], f16, kind="ExternalInput").ap()
    zeros_d = nc.dram_tensor("zeros", [128, M2], f16, kind="ExternalInput").ap()
    yp = nc.dram_tensor("yp", [2, 2, T], f16, kind="ExternalOutput").ap()

    CW = 512

    from contextlib import ExitStack
    with tile.TileContext(nc) as tc, ExitStack() as ctx:
        cpool = ctx.enter_context(tc.tile_pool(name="consts", bufs=1))
        work = ctx.enter_context(tc.tile_pool(name="work", bufs=1))
        psA = ctx.enter_context(tc.tile_pool(name="psA", bufs=2, space="PSUM"))
        psT = ctx.enter_context(tc.tile_pool(name="psT", bufs=1, space="PSUM"))

        mats = cpool.tile([128, NMAT * 3 * 128], f16, tag="mats")
        tabs = cpool.tile([128, NTAB * 2 * M2], f16, tag="tabs")
        htt = cpool.tile([128, 8 * M2], f16, tag="htt")
        idt = cpool.tile([128, 128], f16, tag="idt")
        nc.sync.dma_start(mats[:, 0:3 * 128], mats_d[:, 0:3 * 128])

        def load_consts_small():
            nc.scalar.dma_start(tabs[:], tabs_d[:, :])
            nc.scalar.dma_start(idt[:], id_d[:, :])

        def load_consts_rest():
            nc.sync.dma_start(mats[:, 3 * 128:9 * 3 * 128],
                              mats_d[:, 3 * 128:9 * 3 * 128])
            nc.sync.dma_start(mats[:, 9 * 3 * 128:], mats_d[:, 9 * 3 * 128:])
            nc.scalar.dma_start(htt[:], htab_d[:, :])

        def MAT(m, k):
            return mats[:, (m * 3 + k) * 128:(m * 3 + k + 1) * 128]

        def TAB(t, k):
            return tabs[:, (t * 2 + k) * M2:(t * 2 + k + 1) * M2]

        def HT(o, i, k):
            idx = ((o * 2 + i) * 2 + k)
            return htt[:, idx * M2:(idx + 1) * M2]

        def cmm(pr, pi, m, vr, vi, start, stop):
            """psum += M @ (vr + j vi), complex; M = mats[m] (r/i/-i)."""
            nc.tensor.matmul(pr, MAT(m, 0), vr, start=start, stop=False)
            nc.tensor.matmul(pr, MAT(m, 2), vi, start=False, stop=stop)
            nc.tensor.matmul(pi, MAT(m, 1), vr, start=start, stop=False)
            nc.tensor.matmul(pi, MAT(m, 0), vi, start=False, stop=stop)

        def stt(eng, out, a, b, op):
            """out = a op b. Plain TensorTensor: DVE gets 2x_1p in fp16
            (scalar_tensor_tensor would disable all DVE perf modes)."""
            if op is alu.mult:
                eng.tensor_mul(out, a, b)
            elif op is alu.add:
                eng.tensor_add(out, a, b)
            else:
                eng.tensor_sub(out, a, b)

        # engine rotation for PSUM-exit chunk copies
        cp_state = [0]

        def chunk_copy(dst, src, eng=None):
            """PSUM->SBUF chunk copy, rotating ACT (5) : DVE (1)."""
            i = cp_state[0]
            cp_state[0] += 1
            if i % 9 < 8:
                nc.scalar.copy(dst, src)
            else:
                nc.vector.tensor_copy(dst, src)

        def stage_plain(dst_r, dst_i, src_r, src_i, m):
            """dst = mats[m] @ src (complex), chunked 512; plain copy out."""
            for ch in range(4):
                sl = (slice(None), slice(ch * CW, (ch + 1) * CW))
                pr = psA.tile([128, CW], f32, tag="pr")
                pi = psA.tile([128, CW], f32, tag="pi")
                cmm(pr[:], pi[:], m, src_r[sl], src_i[sl], True, True)
                chunk_copy(dst_r[sl], pr[:])
                chunk_copy(dst_i[sl], pi[:])

        def stage_fold128(dst_r, dst_i, src_r, src_i, m0):
            """dst chunk n3 (128 wide, contiguous) = mats[m0+n3] @ src chunk."""
            for g in range(4):
                pr = psA.tile([128, CW], f32, tag="pr")
                pi = psA.tile([128, CW], f32, tag="pi")
                for q in range(4):
                    n3 = g * 4 + q
                    ssl = (slice(None), slice(n3 * 128, (n3 + 1) * 128))
                    psl = (slice(None), slice(q * 128, (q + 1) * 128))
                    cmm(pr[psl], pi[psl], m0 + n3, src_r[ssl], src_i[ssl],
                        True, True)
                sl = (slice(None), slice(g * CW, (g + 1) * CW))
                chunk_copy(dst_r[sl], pr[:])
                chunk_copy(dst_i[sl], pi[:])

        def cmul(dst_r, dst_i, sr, si, twr, twi):
            """dst = (sr + j si) * (twr + j twi), full-width fp16 STT."""
            c0 = work.tile([128, M2], f16, tag="c0")
            c1 = work.tile([128, M2], f16, tag="c1")
            c3 = work.tile([128, M2], f16, tag="s0")
            stt(nc.gpsimd, c3[:], si[:], twr, alu.mult)
            stt(nc.vector, c0[:], sr[:], twr, alu.mult)
            stt(nc.vector, c1[:], si[:], twi, alu.mult)
            stt(nc.vector, dst_r[:], c0[:], c1[:], alu.subtract)
            c2 = work.tile([128, M2], f16, tag="c0")
            stt(nc.vector, c2[:], sr[:], twi, alu.mult)
            stt(nc.vector, dst_i[:], c2[:], c3[:], alu.add)

        def stage_cmul(dst_r, dst_i, src_r, src_i, m, tw):
            """dst = tw * (mats[m] @ src): matmul, ACT precopy, STT cmul."""
            sr = work.tile([128, M2], f16, tag="sr", bufs=2)
            si = work.tile([128, M2], f16, tag="si", bufs=2)
            for ch in range(4):
                sl = (slice(None), slice(ch * CW, (ch + 1) * CW))
                pr = psA.tile([128, CW], f32, tag="pr")
                pi = psA.tile([128, CW], f32, tag="pi")
                cmm(pr[:], pi[:], m, src_r[sl], src_i[sl], True, True)
                nc.scalar.copy(sr[sl], pr[:])
                nc.scalar.copy(si[sl], pi[:])
            cmul(dst_r, dst_i, sr, si, tw[0], tw[1])

        def cmul_res(dst_r, dst_i, src_r, src_i, m0, tw):
            """inv S2' (tA folded, contiguous n3 chunks) + residual tB cmul."""
            sr = work.tile([128, M2], f16, tag="sr", bufs=2)
            si = work.tile([128, M2], f16, tag="si", bufs=2)
            stage_fold128(sr, si, src_r, src_i, m0)
            cmul(dst_r, dst_i, sr, si, tw[0], tw[1])

        def back_plain(d, pt, hf):
            if hf == 0:
                nc.vector.tensor_copy(d[:, 0:1024], pt[:])
            else:
                nc.scalar.copy(d[:, 1024:M2], pt[:])

        def back_ju(d, pt, hf):
            # psum [k2; j*128 + n3*8+u] -> dst [k2; n3*128 + j*8 + u]
            dd = d[:, :].rearrange("p (n j u) -> p j n u", n=16, j=16, u=8)
            pp = pt[:].rearrange("p (j n u) -> p j n u", j=8, n=16, u=8)
            if hf == 0:
                nc.vector.tensor_copy(dd[:, 0:8], pp)
            else:
                nc.scalar.copy(dd[:, 8:16], pp)

        def tpose16(dst_r, dst_i, src_r, src_i, slicer, back=back_plain):
            """16 PE transposes per plane -> fp16 psum -> 1 copyback.
            Real plane back on DVE, imag plane on ACT (parallel)."""
            for s, d in ((src_r, dst_r), (src_i, dst_i)):
                for hf in range(2):
                    pt = psT.tile([128, 1024], f16, tag="pt", bufs=4,
                                  name="pt_t")
                    for c in range(8):
                        cc = hf * 8 + c
                        nc.tensor.transpose(
                            pt[:, c * 128:(c + 1) * 128], slicer(s, cc), idt[:])
                    back(d, pt, hf)

        def sl_str16(s, c):          # fwd T1 / fwd T2: strided 16
            return s[:, c:M2:16]

        def sl_cont(s, c):           # inv T2' / inv T1': contiguous
            return s[:, c * 128:(c + 1) * 128]

        def fwd_stages(in_r, in_i, vr, vi):
            st = {}

            def s1():
                st['ar'] = work.tile([128, M2], f16, tag="ar", bufs=2, name="ar_t")
                st['ai'] = work.tile([128, M2], f16, tag="ai", bufs=2, name="ai_t")
                stage_cmul(st['ar'], st['ai'], in_r, in_i, 0,
                           (TAB(0, 0), TAB(0, 1)))

            def t1():
                st['btr'] = work.tile([128, M2], f16, tag="btr", bufs=2, name="btr_t")
                st['bti'] = work.tile([128, M2], f16, tag="bti", bufs=2, name="bti_t")
                tpose16(st['btr'], st['bti'], st['ar'], st['ai'], sl_str16)

            def s2():
                st['ar2'] = work.tile([128, M2], f16, tag="ar", bufs=2, name="ar_t")
                st['ai2'] = work.tile([128, M2], f16, tag="ai", bufs=2, name="ai_t")
                stage_fold128(st['ar2'], st['ai2'], st['btr'], st['bti'], 3)

            def t2():
                st['btr2'] = work.tile([128, M2], f16, tag="btr", bufs=2, name="btr_t")
                st['bti2'] = work.tile([128, M2], f16, tag="bti", bufs=2, name="bti_t")
                tpose16(st['btr2'], st['bti2'], st['ar2'], st['ai2'], sl_str16)

            def s3():
                stage_plain(vr, vi, st['btr2'], st['bti2'], 1)

            return [s1, t1, s2, t2, s3]

        def inv_stages(vt, blk, o):
            st = {}

            def sp():
                ssr = work.tile([128, M2], f16, tag="ssr", name="ssr_t")
                ssi = work.tile([128, M2], f16, tag="ssi", name="ssi_t")
                spectral(ssr, ssi, vt, o)
                st['ssr'], st['ssi'] = ssr, ssi

            def s1p():
                st['ar'] = work.tile([128, M2], f16, tag="ar", bufs=2, name="ar_t")
                st['ai'] = work.tile([128, M2], f16, tag="ai", bufs=2, name="ai_t")
                stage_plain(st['ar'], st['ai'], st['ssr'], st['ssi'], 2)

            def t2p():
                st['btr'] = work.tile([128, M2], f16, tag="btr", bufs=2, name="btr_t")
                st['bti'] = work.tile([128, M2], f16, tag="bti", bufs=2, name="bti_t")
                tpose16(st['btr'], st['bti'], st['ar'], st['ai'], sl_cont,
                        back=back_ju)

            def s2p():
                st['ar2'] = work.tile([128, M2], f16, tag="ar", bufs=2, name="ar_t")
                st['ai2'] = work.tile([128, M2], f16, tag="ai", bufs=2, name="ai_t")
                cmul_res(st['ar2'], st['ai2'], st['btr'], st['bti'], 19,
                         (TAB(1, 0), TAB(1, 1)))

            def t1p():
                st['btr2'] = work.tile([128, M2], f16, tag="btr", bufs=2, name="btr_t")
                st['bti2'] = work.tile([128, M2], f16, tag="bti", bufs=2, name="bti_t")
                tpose16(st['btr2'], st['bti2'], st['ar2'], st['ai2'], sl_cont)

            def s3p():
                out_r = work.tile([128, M2], f16, tag="outr")
                out_i = work.tile([128, M2], f16, tag="outi")
                bt_r2, bt_i2 = st['btr2'], st['bti2']
                for g in range(4):
                    pr = psA.tile([128, CW], f32, tag="pr")
                    pi = psA.tile([128, CW], f32, tag="pi")
                    for q in range(4):
                        n3 = g * 4 + q
                        ssl = (slice(None), slice(n3 * 128, (n3 + 1) * 128))
                        psl = (slice(None), slice(q * 128, (q + 1) * 128))
                        cmm(pr[psl], pi[psl], 35 + n3, bt_r2[ssl], bt_i2[ssl],
                            True, True)
                    for d, p in ((out_r, pr), (out_i, pi)):
                        dd = d[:, :].rearrange("p (n2 n3) -> p n3 n2", n3=16)
                        ps = p[:].rearrange("p (q n2) -> p q n2", q=4)
                        nc.scalar.copy(dd[:, g*4:g*4+4, :], ps)
                store_block(out_r, yp[0, o], blk)
                store_block(out_i, yp[1, o], blk)

            return [sp, s1p, t2p, s2p, t1p, s3p]

        def load_block(t_, src, blk, eng):
            if blk == 0:
                eng.dma_start(t_[0:VROW + 1, :], zeros_d[0:VROW + 1, :])
                eng.dma_start(
                    t_[VROW:VROW + 1, VCOL:M2],
                    src[0:ROW_TAIL].rearrange('(a b) -> a b', a=1))
                eng.dma_start(
                    t_[VROW + 1:128, :],
                    src[ROW_TAIL:HOP].rearrange("(r m) -> r m", m=M2))
            elif blk == 1:
                s0 = HOP - (L - 1)
                eng.dma_start(
                    t_[:, :], src[s0:s0 + N].rearrange("(r m) -> r m", m=M2))
            else:
                s0 = 2 * HOP - (L - 1)
                nfull = (T - s0) // M2            # 88
                rem = (T - s0) - nfull * M2       # 1085
                eng.dma_start(t_[nfull:128, :], zeros_d[nfull:128, :])
                eng.dma_start(
                    t_[0:nfull, :],
                    src[s0:s0 + nfull * M2].rearrange("(r m) -> r m", m=M2))
                eng.dma_start(
                    t_[nfull:nfull + 1, 0:rem],
                    src[s0 + nfull * M2:T].rearrange('(a b) -> a b', a=1))

        def store_block(out_t, dst, blk):
            base = blk * HOP
            nc.scalar.dma_start(
                dst[base:base + ROW_TAIL].rearrange('(a b) -> a b', a=1),
                out_t[VROW:VROW + 1, VCOL:M2])
            if blk < 2:
                nc.scalar.dma_start(
                    dst[base + ROW_TAIL:base + HOP].rearrange("(r m) -> r m", m=M2),
                    out_t[VROW + 1:128, :])
            else:
                nrem = T - base - ROW_TAIL
                nfull = nrem // M2                # 44
                rem = nrem - nfull * M2           # 1085
                nc.scalar.dma_start(
                    dst[base + ROW_TAIL:base + ROW_TAIL + nfull * M2]
                        .rearrange("(r m) -> r m", m=M2),
                    out_t[VROW + 1:VROW + 1 + nfull, :])
                nc.scalar.dma_start(
                    dst[T - rem:T].rearrange('(a b) -> a b', a=1),
                    out_t[VROW + 1 + nfull:VROW + 2 + nfull, 0:rem])

        def spectral(sr_, si_, v, o):
            """S_o = V0*H_o0 + V1*H_o1 (fp16, SBUF), accumulated in place."""
            (v0r, v0i), (v1r, v1i) = v
            for h in range(2):
                s = slice(h * 1024, (h + 1) * 1024)
                # real part
                c0 = work.tile([128, M2], f16, tag="s0")
                c1 = work.tile([128, M2], f16, tag="s1")
                stt(nc.gpsimd, c1[:, s], v1i[:, s], HT(o, 1, 1)[:, s], alu.mult)
                stt(nc.vector, sr_[:, s], v0r[:, s], HT(o, 0, 0)[:, s], alu.mult)
                stt(nc.vector, c0[:, s], v0i[:, s], HT(o, 0, 1)[:, s], alu.mult)
                stt(nc.vector, sr_[:, s], sr_[:, s], c0[:, s], alu.subtract)
                stt(nc.vector, c0[:, s], v1r[:, s], HT(o, 1, 0)[:, s], alu.mult)
                stt(nc.vector, sr_[:, s], sr_[:, s], c0[:, s], alu.add)
                stt(nc.vector, sr_[:, s], sr_[:, s], c1[:, s], alu.subtract)
                # imag part
                stt(nc.gpsimd, c1[:, s], v1i[:, s], HT(o, 1, 0)[:, s], alu.mult)
                stt(nc.vector, si_[:, s], v0r[:, s], HT(o, 0, 1)[:, s], alu.mult)
                stt(nc.vector, c0[:, s], v0i[:, s], HT(o, 0, 0)[:, s], alu.mult)
                stt(nc.vector, si_[:, s], si_[:, s], c0[:, s], alu.add)
                stt(nc.vector, c0[:, s], v1r[:, s], HT(o, 1, 1)[:, s], alu.mult)
                stt(nc.vector, si_[:, s], si_[:, s], c0[:, s], alu.add)
                stt(nc.vector, si_[:, s], si_[:, s], c1[:, s], alu.add)

        def dbg_dump(idx, t_):
            if kdbg:
                nc.sync.dma_start(dbgh[idx], t_[:, :])

        # ---- program: stage-interleaved software pipeline ----
        def mk_fwd(blk, i):
            in_r = work.tile([128, M2], f16, tag="inr", bufs=2)
            in_i = work.tile([128, M2], f16, tag="ini", bufs=2)
            ldq = nc.sync if i == 0 else nc.scalar
            load_block(in_r, xp[0, i], blk, ldq)
            load_block(in_i, xp[1, i], blk, ldq)
            vb = 2 if i == 0 else 1
            vr = work.tile([128, M2], f16, tag=f"v{i}r", bufs=vb)
            vi = work.tile([128, M2], f16, tag=f"v{i}i", bufs=vb)
            return fwd_stages(in_r, in_i, vr, vi), (vr, vi)

        def run2(a, b):
            """Interleave two stage lists (either may be None)."""
            a = a or []
            b = b or []
            n = max(len(a), len(b))
            for k in range(n):
                if k < len(a):
                    a[k]()
                if k < len(b):
                    b[k]()

        fa, va = mk_fwd(0, 0)
        load_consts_small()
        fb, vb_ = mk_fwd(0, 1)
        load_consts_rest()
        run2(fa, fb)
        vt = [va, vb_]
        for blk in range(NBLK):
            nxt = blk + 1 < NBLK
            if nxt:
                f0, nv0 = mk_fwd(blk + 1, 0)
                run2(inv_stages(vt, blk, 0), f0)
                f1, nv1 = mk_fwd(blk + 1, 1)
                run2(inv_stages(vt, blk, 1), f1)
                vt = [nv0, nv1]
            else:
                run2(inv_stages(vt, blk, 0), inv_stages(vt, blk, 1))

    nc.compile()
    return nc


def _get_prog():
    global _PROG
    if _PROG is None:
        _PROG = _build_program()
    return _PROG


# ---------------- public entry ----------------
def kernel(x, b, c, U_raw, gamma_raw):
    from concourse import bass_utils

    x16 = np.ascontiguousarray(np.asarray(x).astype(np.float16))
    h = _host_ir(np.asarray(b, np.float64), np.asarray(c, np.float64),
                 np.asarray(U_raw, np.float64), np.asarray(gamma_raw, np.float64))
    htab = _host_htab(h)
    mats_packed, tabs_packed, ident = _consts()
    nc = _get_prog()

    in_maps = []
    for core in range(NCORES):
        in_maps.append({
            "xp": x16[2 * core:2 * core + 2],
            "mats": mats_packed, "tabs": tabs_packed,
            "htab": htab, "ident": ident,
            "zeros": np.zeros((128, M2), np.float16),
        })
    res = bass_utils.run_bass_kernel_spmd(nc, in_maps, core_ids=list(range(NCORES)))
    y = np.empty((16, 2, T), np.float32)
    for core in range(NCORES):
        y[2 * core:2 * core + 2] = res.results[core]["yp"].astype(np.float32)
    return y


# revision 59
# speedup vs baseline: 1.0545x; 1.0525x over previous
"""Trainium2 Bass kernel for nn_FDN_88012469830490.

FDN reverb: IR synthesis (host, tiny 6x6 solves) + FFT convolution
(device) of x (16,2,441000) with the 2x2x88200 IR.

Device algorithm per core (2 batches/core, A=2c, B=2c+1):
  overlap-save conv, FFT N=262144 = 128*128*16, hop 173945, 3 blocks.
  Batch packing: V_i = FFT(x_i^A + j x_i^B) per channel i; spectral
  S_o = V_0 H_o0 + V_1 H_o1 (H = host FFT of IR, scaled 1/512);
  IFFT(S_o) = y_o^A + j y_o^B. 12 FFTs/core, all fp16 matmuls on PE
  with fp32 PSUM; twiddles t2 (fwd), tA and the n3-part of tB (inv)
  folded into 16 per-n3 stationary matrices; remaining twiddles (t1
  fwd, tB-residual inv) as fp16 TensorTensor cmuls (DVE 2x mode, one
  product on GpSimd); PSUM exits on ACT/DVE; output stored fp16 and
  widened on host. Stage-interleaved software pipeline: two FFT chains
  in flight (inv(b,o) interleaved with fwd(b+1,o)) over double-buffered
  tiles; x loads/consts split across SP/ACT DMA queues ahead of the
  big constant tables.

Layouts (digits: n = n1*2048+n2*16+n3, k = k1+128*k2+16384*k3,
k1 = 16*u+j):
  fwd: [n1; n2*16+n3] -S1-> [k1; m] -t1-> -T1-> [n2; n3*128+k1]
       -S2(t2 fold)-> [k2; n3*128+k1] -T2-> [n3*8+u; j*128+k2]
       -S3-> [k3*8+u; j*128+k2]
  inv: -S1'-> [n3*8+u; j*128+k2] -T2'-> [k2; j*128+n3*8+u]
       -S2'(tA fold)-> [n2; ...] -tBres-> -T1'-> [j*8+u; n3*128+n2]
       -S3'(n3-fold, 1/512)-> [n1; n3*128+n2] -perm-> [n1; n2*16+n3]
"""
import sys
import numpy as np

sys.path.insert(0, "/opt/trn_rl_repo")

# ---------------- problem constants ----------------
SR = 44100
DELAYS = np.array([997, 1153, 1327, 1559, 1801, 2099])
ND = 6
L = 88200
FB = L // 2 + 1
NDF = 49
T60 = 1.5
GAMMA_MAX = 10.0 ** ((-60.0 / SR / T60 * DELAYS) / 20.0)

T = 441000
N = 262144
P1 = 128
M2 = 2048
HOP = N - (L - 1)     # 173945
NBLK = 3
NCORES = 8

VROW, VCOL = 43, 135  # L-1 = 88199 = 43*2048 + 135
ROW_TAIL = M2 - VCOL              # 1913
NMAT = 51
NTAB = 2


# ---------------- host IR synthesis ----------------
def _expm_skew(S):
    lam, V = np.linalg.eigh(1j * S)
    return (V @ np.diag(np.exp(-1j * lam)) @ V.conj().T).real


def _host_ir(b, c, U_raw, gamma_raw):
    """IR h (2, 2, L) float64."""
    tri = np.triu(U_raw.astype(np.float64), 1)
    U = _expm_skew(tri - tri.T)
    gamma = (1.0 / (1.0 + np.exp(-gamma_raw.astype(np.float64)))) * GAMMA_MAX
    pos = np.arange(FB) * ((NDF - 1) / (FB - 1))
    i0 = np.clip(np.floor(pos).astype(int), 0, NDF - 2)
    frac = (pos - i0)[:, None]
    g = gamma[i0] * (1 - frac) + gamma[i0 + 1] * frac
    A = U[None, :, :] * g[:, None, :]
    freqs = np.arange(FB) / L * 2 * np.pi
    invD = np.exp(1j * freqs[:, None] * DELAYS)
    Mm = invD[:, :, None] * np.eye(ND) - A
    bc = np.broadcast_to(b.astype(np.float64), (FB, ND, 2))
    X = np.linalg.solve(Mm, bc)
    H = np.einsum('ci,fio->fco', c.astype(complex), X)
    h = np.fft.irfft(H.transpose(1, 2, 0), n=L)
    return h


def _kmap():
    p = np.arange(128)[:, None]
    f = np.arange(M2)[None, :]
    k3, u = p // 8, p % 8
    j, k2 = f // 128, f % 128
    k1 = 16 * u + j
    return k1 + 128 * k2 + 16384 * k3


def _host_htab(h):
    """H tables (2,2,2,128,2048) fp16: [o,i,(r,i)] spectra / 512."""
    km = _kmap()
    out = np.empty((128, 8 * M2), np.float16)
    for o in range(2):
        for i in range(2):
            hp = np.zeros(N)
            hp[:L] = h[o, i]
            Hf = np.fft.fft(hp) / 512.0
            idx = (o * 2 + i) * 2
            out[:, idx * M2:(idx + 1) * M2] = Hf[km].real.astype(np.float16)
            out[:, (idx + 1) * M2:(idx + 2) * M2] = Hf[km].imag.astype(np.float16)
    return out


def _consts():
    """Stationary matrices + twiddle tables, host-packed partition-major.

    mats_packed: (128, NMAT*3*128) fp16 — mat idx m, comp k (r/i/-i):
      cols [ (m*3+k)*128 : +128 ].
    tabs_packed: (128, NTAB*2*2048) fp16 — t1(r,i), tBres(r,i).
    """
    q = np.arange(128)
    k1q = 16 * (q % 8) + q // 8

    mats = np.zeros((NMAT, 128, 128), complex)
    # 0: F128 fwd (S1): F[k1, n1] = W^-
    mats[0] = np.exp(-2j * np.pi * np.outer(np.arange(128), np.arange(128)) / 128)
    # 1: Btil fwd (S3): [k3*8+u, n3*8+u] = exp(-2pi i n3 k3/16)
    for u in range(8):
        n3g, k3g = np.meshgrid(np.arange(16), np.arange(16), indexing='xy')
        mats[1][k3g * 8 + u, n3g * 8 + u] = np.exp(-2j * np.pi * n3g * k3g / 16)
        # 2: Btil' inv (S1'): [n3*8+u, k3*8+u] = exp(+2pi i n3 k3/16)
        mats[2][n3g * 8 + u, k3g * 8 + u] = np.exp(2j * np.pi * n3g * k3g / 16)
    # 3..18: fwd S2 with t2 fold: M[n3][k2, n2] = W2048^{-k2 n3} * F128^-[k2,n2]
    F = np.exp(-2j * np.pi * np.outer(np.arange(128), np.arange(128)) / 128)
    for n3 in range(16):
        d = np.exp(-2j * np.pi * np.arange(128) * n3 / M2)
        mats[3 + n3] = d[:, None] * F
    # 19..34: inv S2' with tA fold: A[n3][n2, k2] = F128^+[n2,k2] * W2048^{+n3 k2}
    Fp = np.exp(2j * np.pi * np.outer(np.arange(128), np.arange(128)) / 128)
    for n3 in range(16):
        d = np.exp(2j * np.pi * np.arange(128) * n3 / M2)
        mats[19 + n3] = Fp * d[None, :]
    # 35..50: inv S3' with n3-part of tB + 1/512:
    # P[n3][n1, q] = exp(+2pi i k1(q) (2048*n1 + n3)/N) / 512
    for n3 in range(16):
        mats[35 + n3] = np.exp(
            2j * np.pi * k1q[None, :] * (2048 * np.arange(128)[:, None] + n3) / N
        ) / 512.0

    # matmul computes lhsT.T @ rhs -> store each stationary TRANSPOSED
    mats_packed = np.empty((128, NMAT * 3 * 128), np.float16)
    for m in range(NMAT):
        mt = mats[m].T
        mats_packed[:, (m * 3 + 0) * 128:(m * 3 + 1) * 128] = mt.real.astype(np.float16)
        mats_packed[:, (m * 3 + 1) * 128:(m * 3 + 2) * 128] = mt.imag.astype(np.float16)
        mats_packed[:, (m * 3 + 2) * 128:(m * 3 + 3) * 128] = (-mt.imag).astype(np.float16)

    # tables
    t1 = np.exp(-2j * np.pi * np.outer(np.arange(128), np.arange(M2)) / N)
    # tBres layout [n2; n3*128 + j*8 + u], k1 = 16u + j
    f = np.arange(M2)
    rem = f % 128
    jf, uf = rem // 8, rem % 8
    k1f = 16 * uf + jf
    tb = np.exp(2j * np.pi * np.outer(np.arange(128), k1f) / 16384.0)
    tabs_packed = np.empty((128, NTAB * 2 * M2), np.float16)
    tabs_packed[:, 0*M2:1*M2] = t1.real.astype(np.float16)
    tabs_packed[:, 1*M2:2*M2] = t1.imag.astype(np.float16)
    tabs_packed[:, 2*M2:3*M2] = tb.real.astype(np.float16)
    tabs_packed[:, 3*M2:4*M2] = tb.imag.astype(np.float16)

    ident = np.eye(128, dtype=np.float16)
    return mats_packed, tabs_packed, ident


# ---------------- bass program ----------------
_PROG = None


def _build_program():
    import concourse.bass as bass
    import concourse.tile as tile
    from concourse import bacc, mybir

    f32 = mybir.dt.float32
    f16 = mybir.dt.float16
    alu = mybir.AluOpType
    nc = bacc.Bacc("TRN2", target_bir_lowering=False, debug=False,
                   enable_asserts=False, num_devices=NCORES)

    import os
    kdbg = bool(os.environ.get("KDBG"))
    xp = nc.dram_tensor("xp", [2, 2, T], f16, kind="ExternalInput").ap()
    if kdbg:
        dbgh = nc.dram_tensor("dbgh", [16, 128, M2], f16, kind="ExternalOutput").ap()
    mats_d = nc.dram_tensor("mats", [128, NMAT * 3 * 128], f16, kind="ExternalInput").ap()
    tabs_d = nc.dram_tensor("tabs", [128, NTAB * 2 * M2], f16, kind="ExternalInput").ap()
    htab_d = nc.dram_tensor("htab", [128, 8 * M2], f16, kind="ExternalInput").ap()
    id_d = nc.dram_tensor("ident", [128, 128], f16, kind="ExternalInput").ap()
    zeros_d = nc.dram_tensor("zeros", [128, M2], f16, kind="ExternalInput").ap()
    yp = nc.dram_tensor("yp", [2, 2, T], f16, kind="ExternalOutput").ap()

    CW = 512

    from contextlib import ExitStack
    with tile.TileContext(nc) as tc, ExitStack() as ctx:
        cpool = ctx.enter_context(tc.tile_pool(name="consts", bufs=1))
        work = ctx.enter_context(tc.tile_pool(name="work", bufs=1))
        psA = ctx.enter_context(tc.tile_pool(name="psA", bufs=2, space="PSUM"))
        psT = ctx.enter_context(tc.tile_pool(name="psT", bufs=1, space="PSUM"))

        mats = cpool.tile([128, NMAT * 3 * 128], f16, tag="mats")
        tabs = cpool.tile([128, NTAB * 2 * M2], f16, tag="tabs")
        htt = cpool.tile([128, 8 * M2], f16, tag="htt")
        idt = cpool.tile([128, 128], f16, tag="idt")
        nc.sync.dma_start(mats[:, 0:3 * 128], mats_d[:, 0:3 * 128])

        def load_consts_small():
            nc.scalar.dma_start(tabs[:, 0:2 * M2], tabs_d[:, 0:2 * M2])
            nc.scalar.dma_start(idt[:], id_d[:, :])

        def load_consts_rest():
            nc.scalar.dma_start(tabs[:, 2 * M2:], tabs_d[:, 2 * M2:])
            nc.sync.dma_start(mats[:, 3 * 128:9 * 3 * 128],
                              mats_d[:, 3 * 128:9 * 3 * 128])
            nc.sync.dma_start(mats[:, 9 * 3 * 128:], mats_d[:, 9 * 3 * 128:])
            nc.scalar.dma_start(htt[:], htab_d[:, :])

        def MAT(m, k):
            return mats[:, (m * 3 + k) * 128:(m * 3 + k + 1) * 128]

        def TAB(t, k):
            return tabs[:, (t * 2 + k) * M2:(t * 2 + k + 1) * M2]

        def HT(o, i, k):
            idx = ((o * 2 + i) * 2 + k)
            return htt[:, idx * M2:(idx + 1) * M2]

        def cmm(pr, pi, m, vr, vi, start, stop):
            """psum += M @ (vr + j vi), complex; M = mats[m] (r/i/-i)."""
            nc.tensor.matmul(pr, MAT(m, 0), vr, start=start, stop=False)
            nc.tensor.matmul(pr, MAT(m, 2), vi, start=False, stop=stop)
            nc.tensor.matmul(pi, MAT(m, 1), vr, start=start, stop=False)
            nc.tensor.matmul(pi, MAT(m, 0), vi, start=False, stop=stop)

        def stt(eng, out, a, b, op):
            """out = a op b. Plain TensorTensor: DVE gets 2x_1p in fp16
            (scalar_tensor_tensor would disable all DVE perf modes)."""
            if op is alu.mult:
                eng.tensor_mul(out, a, b)
            elif op is alu.add:
                eng.tensor_add(out, a, b)
            else:
                eng.tensor_sub(out, a, b)

        # engine rotation for PSUM-exit chunk copies
        cp_state = [0]

        def chunk_copy(dst, src, eng=None):
            """PSUM->SBUF chunk copy, rotating ACT (5) : DVE (1)."""
            i = cp_state[0]
            cp_state[0] += 1
            if i % 13 < 12:
                nc.scalar.copy(dst, src)
            else:
                nc.vector.tensor_copy(dst, src)

        def stage_plain(dst_r, dst_i, src_r, src_i, m):
            """dst = mats[m] @ src (complex), chunked 512; plain copy out."""
            for ch in range(4):
                sl = (slice(None), slice(ch * CW, (ch + 1) * CW))
                pr = psA.tile([128, CW], f32, tag="pr")
                pi = psA.tile([128, CW], f32, tag="pi")
                cmm(pr[:], pi[:], m, src_r[sl], src_i[sl], True, True)
                chunk_copy(dst_r[sl], pr[:])
                chunk_copy(dst_i[sl], pi[:])

        def stage_fold128(dst_r, dst_i, src_r, src_i, m0):
            """dst chunk n3 (128 wide, contiguous) = mats[m0+n3] @ src chunk."""
            for g in range(4):
                pr = psA.tile([128, CW], f32, tag="pr")
                pi = psA.tile([128, CW], f32, tag="pi")
                for q in range(4):
                    n3 = g * 4 + q
                    ssl = (slice(None), slice(n3 * 128, (n3 + 1) * 128))
                    psl = (slice(None), slice(q * 128, (q + 1) * 128))
                    cmm(pr[psl], pi[psl], m0 + n3, src_r[ssl], src_i[ssl],
                        True, True)
                sl = (slice(None), slice(g * CW, (g + 1) * CW))
                chunk_copy(dst_r[sl], pr[:])
                chunk_copy(dst_i[sl], pi[:])

        def cmul(dst_r, dst_i, sr, si, twr, twi):
            """dst = (sr + j si) * (twr + j twi), full-width fp16 STT."""
            c0 = work.tile([128, M2], f16, tag="c0")
            c1 = work.tile([128, M2], f16, tag="c1")
            c3 = work.tile([128, M2], f16, tag="s0")
            stt(nc.gpsimd, c3[:], si[:], twr, alu.mult)
            stt(nc.vector, c0[:], sr[:], twr, alu.mult)
            stt(nc.vector, c1[:], si[:], twi, alu.mult)
            stt(nc.vector, dst_r[:], c0[:], c1[:], alu.subtract)
            c2 = work.tile([128, M2], f16, tag="c0")
            stt(nc.vector, c2[:], sr[:], twi, alu.mult)
            stt(nc.vector, dst_i[:], c2[:], c3[:], alu.add)

        def stage_cmul(dst_r, dst_i, src_r, src_i, m, tw):
            """dst = tw * (mats[m] @ src): matmul, ACT precopy, STT cmul."""
            sr = work.tile([128, M2], f16, tag="sr", bufs=2)
            si = work.tile([128, M2], f16, tag="si", bufs=2)
            for ch in range(4):
                sl = (slice(None), slice(ch * CW, (ch + 1) * CW))
                pr = psA.tile([128, CW], f32, tag="pr")
                pi = psA.tile([128, CW], f32, tag="pi")
                cmm(pr[:], pi[:], m, src_r[sl], src_i[sl], True, True)
                nc.scalar.copy(sr[sl], pr[:])
                nc.scalar.copy(si[sl], pi[:])
            cmul(dst_r, dst_i, sr, si, tw[0], tw[1])

        def cmul_res(dst_r, dst_i, src_r, src_i, m0, tw):
            """inv S2' (tA folded, contiguous n3 chunks) + residual tB cmul."""
            sr = work.tile([128, M2], f16, tag="sr", bufs=2)
            si = work.tile([128, M2], f16, tag="si", bufs=2)
            stage_fold128(sr, si, src_r, src_i, m0)
            cmul(dst_r, dst_i, sr, si, tw[0], tw[1])

        def back_plain(d, pt, hf):
            if hf == 0:
                nc.vector.tensor_copy(d[:, 0:1024], pt[:])
            else:
                nc.scalar.copy(d[:, 1024:M2], pt[:])

        def back_ju(d, pt, hf):
            # psum [k2; j*128 + n3*8+u] -> dst [k2; n3*128 + j*8 + u]
            dd = d[:, :].rearrange("p (n j u) -> p j n u", n=16, j=16, u=8)
            pp = pt[:].rearrange("p (j n u) -> p j n u", j=8, n=16, u=8)
            if hf == 0:
                nc.vector.tensor_copy(dd[:, 0:8], pp)
            else:
                nc.scalar.copy(dd[:, 8:16], pp)

        def tpose16(dst_r, dst_i, src_r, src_i, slicer, back=back_plain):
            """16 PE transposes per plane -> fp16 psum -> 1 copyback.
            Real plane back on DVE, imag plane on ACT (parallel)."""
            for s, d in ((src_r, dst_r), (src_i, dst_i)):
                for hf in range(2):
                    pt = psT.tile([128, 1024], f16, tag="pt", bufs=4,
                                  name="pt_t")
                    for c in range(8):
                        cc = hf * 8 + c
                        nc.tensor.transpose(
                            pt[:, c * 128:(c + 1) * 128], slicer(s, cc), idt[:])
                    back(d, pt, hf)

        def sl_str16(s, c):          # fwd T1 / fwd T2: strided 16
            return s[:, c:M2:16]

        def sl_cont(s, c):           # inv T2' / inv T1': contiguous
            return s[:, c * 128:(c + 1) * 128]

        def fwd_stages(in_r, in_i, vr, vi):
            st = {}

            def s1():
                st['ar'] = work.tile([128, M2], f16, tag="ar", bufs=2, name="ar_t")
                st['ai'] = work.tile([128, M2], f16, tag="ai", bufs=2, name="ai_t")
                stage_cmul(st['ar'], st['ai'], in_r, in_i, 0,
                           (TAB(0, 0), TAB(0, 1)))

            def t1():
                st['btr'] = work.tile([128, M2], f16, tag="btr", bufs=2, name="btr_t")
                st['bti'] = work.tile([128, M2], f16, tag="bti", bufs=2, name="bti_t")
                tpose16(st['btr'], st['bti'], st['ar'], st['ai'], sl_str16)

            def s2():
                st['ar2'] = work.tile([128, M2], f16, tag="ar", bufs=2, name="ar_t")
                st['ai2'] = work.tile([128, M2], f16, tag="ai", bufs=2, name="ai_t")
                stage_fold128(st['ar2'], st['ai2'], st['btr'], st['bti'], 3)

            def t2():
                st['btr2'] = work.tile([128, M2], f16, tag="btr", bufs=2, name="btr_t")
                st['bti2'] = work.tile([128, M2], f16, tag="bti", bufs=2, name="bti_t")
                tpose16(st['btr2'], st['bti2'], st['ar2'], st['ai2'], sl_str16)

            def s3():
                stage_plain(vr, vi, st['btr2'], st['bti2'], 1)

            return [s1, t1, s2, t2, s3]

        def inv_stages(vt, blk, o):
            st = {}

            def sp():
                ssr = work.tile([128, M2], f16, tag="ssr", name="ssr_t")
                ssi = work.tile([128, M2], f16, tag="ssi", name="ssi_t")
                spectral(ssr, ssi, vt, o)
                st['ssr'], st['ssi'] = ssr, ssi

            def s1p():
                st['ar'] = work.tile([128, M2], f16, tag="ar", bufs=2, name="ar_t")
                st['ai'] = work.tile([128, M2], f16, tag="ai", bufs=2, name="ai_t")
                stage_plain(st['ar'], st['ai'], st['ssr'], st['ssi'], 2)

            def t2p():
                st['btr'] = work.tile([128, M2], f16, tag="btr", bufs=2, name="btr_t")
                st['bti'] = work.tile([128, M2], f16, tag="bti", bufs=2, name="bti_t")
                tpose16(st['btr'], st['bti'], st['ar'], st['ai'], sl_cont,
                        back=back_ju)

            def s2p():
                st['ar2'] = work.tile([128, M2], f16, tag="ar", bufs=2, name="ar_t")
                st['ai2'] = work.tile([128, M2], f16, tag="ai", bufs=2, name="ai_t")
                cmul_res(st['ar2'], st['ai2'], st['btr'], st['bti'], 19,
                         (TAB(1, 0), TAB(1, 1)))

            def t1p():
                st['btr2'] = work.tile([128, M2], f16, tag="btr", bufs=2, name="btr_t")
                st['bti2'] = work.tile([128, M2], f16, tag="bti", bufs=2, name="bti_t")
                tpose16(st['btr2'], st['bti2'], st['ar2'], st['ai2'], sl_cont)

            def s3p():
                out_r = work.tile([128, M2], f16, tag="outr")
                out_i = work.tile([128, M2], f16, tag="outi")
                bt_r2, bt_i2 = st['btr2'], st['bti2']
                for g in range(4):
                    pr = psA.tile([128, CW], f32, tag="pr")
                    pi = psA.tile([128, CW], f32, tag="pi")
                    for q in range(4):
                        n3 = g * 4 + q
                        ssl = (slice(None), slice(n3 * 128, (n3 + 1) * 128))
                        psl = (slice(None), slice(q * 128, (q + 1) * 128))
                        cmm(pr[psl], pi[psl], 35 + n3, bt_r2[ssl], bt_i2[ssl],
                            True, True)
                    for d, p in ((out_r, pr), (out_i, pi)):
                        dd = d[:, :].rearrange("p (n2 n3) -> p n3 n2", n3=16)
                        ps = p[:].rearrange("p (q n2) -> p q n2", q=4)
                        nc.scalar.copy(dd[:, g*4:g*4+4, :], ps)
                store_block(out_r, yp[0, o], blk)
                store_block(out_i, yp[1, o], blk)

            return [sp, s1p, t2p, s2p, t1p, s3p]

        def load_block(t_, src, blk, eng):
            if blk == 0:
                eng.dma_start(t_[0:VROW + 1, :], zeros_d[0:VROW + 1, :])
                eng.dma_start(
                    t_[VROW:VROW + 1, VCOL:M2],
                    src[0:ROW_TAIL].rearrange('(a b) -> a b', a=1))
                eng.dma_start(
                    t_[VROW + 1:128, :],
                    src[ROW_TAIL:HOP].rearrange("(r m) -> r m", m=M2))
            elif blk == 1:
                s0 = HOP - (L - 1)
                eng.dma_start(
                    t_[:, :], src[s0:s0 + N].rearrange("(r m) -> r m", m=M2))
            else:
                s0 = 2 * HOP - (L - 1)
                nfull = (T - s0) // M2            # 88
                rem = (T - s0) - nfull * M2       # 1085
                eng.dma_start(t_[nfull:128, :], zeros_d[nfull:128, :])
                eng.dma_start(
                    t_[0:nfull, :],
                    src[s0:s0 + nfull * M2].rearrange("(r m) -> r m", m=M2))
                eng.dma_start(
                    t_[nfull:nfull + 1, 0:rem],
                    src[s0 + nfull * M2:T].rearrange('(a b) -> a b', a=1))

        def store_block(out_t, dst, blk):
            base = blk * HOP
            nc.scalar.dma_start(
                dst[base:base + ROW_TAIL].rearrange('(a b) -> a b', a=1),
                out_t[VROW:VROW + 1, VCOL:M2])
            if blk < 2:
                nc.scalar.dma_start(
                    dst[base + ROW_TAIL:base + HOP].rearrange("(r m) -> r m", m=M2),
                    out_t[VROW + 1:128, :])
            else:
                nrem = T - base - ROW_TAIL
                nfull = nrem // M2                # 44
                rem = nrem - nfull * M2           # 1085
                nc.scalar.dma_start(
                    dst[base + ROW_TAIL:base + ROW_TAIL + nfull * M2]
                        .rearrange("(r m) -> r m", m=M2),
                    out_t[VROW + 1:VROW + 1 + nfull, :])
                nc.scalar.dma_start(
                    dst[T - rem:T].rearrange('(a b) -> a b', a=1),
                    out_t[VROW + 1 + nfull:VROW + 2 + nfull, 0:rem])

        def spectral(sr_, si_, v, o):
            """S_o = V0*H_o0 + V1*H_o1 (fp16, SBUF), accumulated in place."""
            (v0r, v0i), (v1r, v1i) = v
            for h in range(2):
                s = slice(h * 1024, (h + 1) * 1024)
                # real part
                c0 = work.tile([128, M2], f16, tag="s0")
                c1 = work.tile([128, M2], f16, tag="s1")
                stt(nc.gpsimd, c1[:, s], v1i[:, s], HT(o, 1, 1)[:, s], alu.mult)
                stt(nc.vector, sr_[:, s], v0r[:, s], HT(o, 0, 0)[:, s], alu.mult)
                stt(nc.vector, c0[:, s], v0i[:, s], HT(o, 0, 1)[:, s], alu.mult)
                stt(nc.vector, sr_[:, s], sr_[:, s], c0[:, s], alu.subtract)
                stt(nc.vector, c0[:, s], v1r[:, s], HT(o, 1, 0)[:, s], alu.mult)
                stt(nc.vector, sr_[:, s], sr_[:, s], c0[:, s], alu.add)
                stt(nc.vector, sr_[:, s], sr_[:, s], c1[:, s], alu.subtract)
                # imag part
                stt(nc.gpsimd, c1[:, s], v1i[:, s], HT(o, 1, 0)[:, s], alu.mult)
                stt(nc.vector, si_[:, s], v0r[:, s], HT(o, 0, 1)[:, s], alu.mult)
                stt(nc.vector, c0[:, s], v0i[:, s], HT(o, 0, 0)[:, s], alu.mult)
                stt(nc.vector, si_[:, s], si_[:, s], c0[:, s], alu.add)
                stt(nc.vector, c0[:, s], v1r[:, s], HT(o, 1, 1)[:, s], alu.mult)
                stt(nc.vector, si_[:, s], si_[:, s], c0[:, s], alu.add)
                stt(nc.vector, si_[:, s], si_[:, s], c1[:, s], alu.add)

        def dbg_dump(idx, t_):
            if kdbg:
                nc.sync.dma_start(dbgh[idx], t_[:, :])

        # ---- program: stage-interleaved software pipeline ----
        def mk_fwd(blk, i):
            in_r = work.tile([128, M2], f16, tag="inr", bufs=2)
            in_i = work.tile([128, M2], f16, tag="ini", bufs=2)
            ldq = nc.sync if i == 0 else nc.scalar
            load_block(in_r, xp[0, i], blk, ldq)
            load_block(in_i, xp[1, i], blk, ldq)
            vb = 2 if i == 0 else 1
            vr = work.tile([128, M2], f16, tag=f"v{i}r", bufs=vb)
            vi = work.tile([128, M2], f16, tag=f"v{i}i", bufs=vb)
            return fwd_stages(in_r, in_i, vr, vi), (vr, vi)

        def run2(a, b):
            """Interleave two stage lists (either may be None)."""
            a = a or []
            b = b or []
            n = max(len(a), len(b))
            for k in range(n):
                if k < len(a):
                    a[k]()
                if k < len(b):
                    b[k]()

        fa, va = mk_fwd(0, 0)
        load_consts_small()
        fb, vb_ = mk_fwd(0, 1)
        load_consts_rest()
        run2(fa, fb)
        vt = [va, vb_]
        for blk in range(NBLK):
            nxt = blk + 1 < NBLK
            if nxt:
                f0, nv0 = mk_fwd(blk + 1, 0)
                run2(inv_stages(vt, blk, 0), f0)
                f1, nv1 = mk_fwd(blk + 1, 1)
                run2(inv_stages(vt, blk, 1), f1)
                vt = [nv0, nv1]
            else:
                run2(inv_stages(vt, blk, 0), inv_stages(vt, blk, 1))

    nc.compile()
    return nc


def _get_prog():
    global _PROG
    if _PROG is None:
        _PROG = _build_program()
    return _PROG


# ---------------- public entry ----------------
def kernel(x, b, c, U_raw, gamma_raw):
    from concourse import bass_utils

    x16 = np.ascontiguousarray(np.asarray(x).astype(np.float16))
    h = _host_ir(np.asarray(b, np.float64), np.asarray(c, np.float64),
                 np.asarray(U_raw, np.float64), np.asarray(gamma_raw, np.float64))
    htab = _host_htab(h)
    mats_packed, tabs_packed, ident = _consts()
    nc = _get_prog()

    in_maps = []
    for core in range(NCORES):
        in_maps.append({
            "xp": x16[2 * core:2 * core + 2],
            "mats": mats_packed, "tabs": tabs_packed,
            "htab": htab, "ident": ident,
            "zeros": np.zeros((128, M2), np.float16),
        })
    res = bass_utils.run_bass_kernel_spmd(nc, in_maps, core_ids=list(range(NCORES)))
    y = np.empty((16, 2, T), np.float32)
    for core in range(NCORES):
        y[2 * core:2 * core + 2] = res.results[core]["yp"].astype(np.float32)
    return y
